# revision 52
# baseline (speedup 1.0000x reference)
"""Lifted-structure smoothed metric loss on 8 Trainium2 NeuronCores.

Strategy (cyclic-band symmetric version):
  - Two phases: tt (text [4096,256]) and st (mixed emb [8192,256]).
  - Each core owns N/8 rows. rhs buffers are column-rotated per core so the
    core's own diagonal block sits at local column 0. Exploiting symmetry of
    the distance matrix, each core computes E = exp(m - D) only for a cyclic
    band of 5*R local columns (R = rows/core): blocks c..c+3 fully, plus the
    antipodal block c+4 which cores 4-7 fully mask out (covered by cores 0-3).
  - d2/2 comes from bf16 matmuls (K=128+128 data + K=65 padded aug rows
    carrying -sq_j/2 as bf16 hi+lo vs an all-ones stationary pair; K=65
    keeps the PE tile_size at 128x128 so matmuls pipeline at full rate).
  - VectorE drains PSUM into fp16 SBUF tiles adding the exact fp32 -sq_i/2.
  - ScalarE: D = Sqrt(-2*S), then E = Exp(-D + margin) with hardware row-sum
    accumulation (the band sum). Table phases grouped to amortize ACT loads.
  - The transposed contributions (rowsum_j for in-band pairs) are column sums
    of E, computed on the PE as ones[128,65].T @ E into small PSUM tiles,
    drained/accumulated by VectorE into per-core column-sum vectors.
  - Masking matmuls (identity x pattern) suppress diagonal/pair/duplicate
    entries; mask values are per-core inputs so the program stays SPMD.
  - Host: scatters band sums + column sums into global rowsums, adds back the
    exact duplicate terms, and runs the O(N) float64 loss epilogue.
"""
import sys

sys.path.insert(0, "/opt/trn_rl_repo")
sys.path.insert(0, "/opt/pypackages")

from contextlib import ExitStack

import ml_dtypes
import numpy as np

import concourse.bass as bass
import concourse.tile as tile
from concourse import bacc, mybir
from concourse.bass_utils import run_bass_kernel_spmd
from concourse.tile_rust import add_dep_helper

f32 = mybir.dt.float32
f16 = mybir.dt.float16
f8 = mybir.dt.float8e4
bf16 = mybir.dt.bfloat16
bf = ml_dtypes.bfloat16

N_TT, N_ST, DIM, CORES = 4096, 8192, 256, 8
MARGIN = 1.0
ROWS_TT, ROWS_ST = N_TT // CORES, N_ST // CORES      # 512, 1024
BLKS_TT, BLKS_ST = ROWS_TT // 128, ROWS_ST // 128    # 4, 8
BAND_TT, BAND_ST = 5 * ROWS_TT, 5 * ROWS_ST          # 2560, 5120
WIN = 1024
MASK_VAL = -6144.0                                   # S += MASK_VAL -> d2 += 12288
NBLK = BLKS_TT + BLKS_ST                             # 12

_prog_cache = {}

# table-phase groups of blocks: [tt x4], st pairs, last pair split so the
# final colsum matmuls (which serialize after the last exp) cover one block
_GROUPS = ([[("tt", b) for b in range(BLKS_TT)]] +
           [[("st", 2 * k), ("st", 2 * k + 1)]
            for k in range(BLKS_ST // 2 - 1)] +
           [[("st", BLKS_ST - 2)], [("st", BLKS_ST - 1)]])


def _build_program():
    nc = bacc.Bacc("TRN2", target_bir_lowering=False, debug=False,
                   enable_asserts=False)

    ins = {}
    for name, shape, dt in [
            ("ones2", [65, 128], bf16), ("ones_cs", [128, 65], bf16),
            ("ident", [128, 128], bf16),
            ("mdiag", [128, 128], bf16), ("mdup", [128, 128], bf16),
            ("am", [128, 512], bf16),
            ("sqi", [128, NBLK], f32),
            ("r0_tt", [128, N_TT], bf16), ("r1_tt", [128, N_TT], bf16),
            ("augr_tt", [65, N_TT], bf16),
            ("r0_st", [128, N_ST], bf16), ("r1_st", [128, N_ST], bf16),
            ("augr_st", [65, N_ST], bf16)]:
        ins[name] = nc.dram_tensor(name, shape, dt, kind="ExternalInput")

    out_acc = nc.dram_tensor("acc", [(NBLK + 4) * 128, 1], f32,
                             kind="ExternalOutput")
    out_ctt = nc.dram_tensor("ctt", [BLKS_TT, BAND_TT - ROWS_TT], f32,
                             kind="ExternalOutput")
    out_cst = nc.dram_tensor("cst", [BLKS_ST, BAND_ST - ROWS_ST], f32,
                             kind="ExternalOutput")

    with tile.TileContext(nc) as tc, ExitStack() as ctx:
        sb = ctx.enter_context(tc.tile_pool(name="sb", bufs=1))
        stt_p = ctx.enter_context(tc.tile_pool(name="stt", bufs=4))
        sst_p = ctx.enter_context(tc.tile_pool(name="sst", bufs=3))
        dtt_p = ctx.enter_context(tc.tile_pool(name="dtt", bufs=4))
        dst_p = ctx.enter_context(tc.tile_pool(name="dst", bufs=2))
        ett_p = ctx.enter_context(tc.tile_pool(name="ett", bufs=2))
        est_p = ctx.enter_context(tc.tile_pool(name="est", bufs=2))
        cstg_p = ctx.enter_context(tc.tile_pool(name="cstg", bufs=3))
        pspool = ctx.enter_context(tc.tile_pool(name="ps", bufs=2, space="PSUM"))
        cpspool = ctx.enter_context(tc.tile_pool(name="cps", bufs=2, space="PSUM"))

        sbt = {}
        H_TT = N_TT // 2
        small = ("ones2", "ones_cs", "ident", "mdiag", "mdup", "am", "sqi")
        for name in small:
            t = ins[name]
            st_tile = sb.tile(list(t.shape), t.dtype, tag=name)
            # gpsimd (SW-DGE) queue: loads tiny consts in parallel with the
            # big input loads on the sync queue, so the first matmul starts
            # as soon as the first tt chunks land
            nc.gpsimd.dma_start(st_tile[:], t.ap())
            sbt[name] = st_tile
        # tt inputs split in column halves so the first matmuls start early
        for half, c0 in (("a", 0), ("b", H_TT)):
            for name in ("r0_tt", "r1_tt", "augr_tt"):
                t = ins[name]
                st_tile = sb.tile([t.shape[0], H_TT], t.dtype,
                                  tag=f"{name}_{half}", name=f"{name}_{half}")
                nc.sync.dma_start(st_tile[:], t.ap()[:, c0:c0 + H_TT])
                sbt[f"{name}_{half}"] = st_tile
        for name in ("r0_st", "r1_st", "augr_st"):
            t = ins[name]
            st_tile = sb.tile(list(t.shape), t.dtype, tag=name)
            nc.sync.dma_start(st_tile[:], t.ap())
            sbt[name] = st_tile

        # cols 0..NBLK-1: per-block band sums; cols NBLK..NBLK+3: the two
        # chunked tail blocks' partial band sums (2 chunks each)
        acc = sb.tile([128, NBLK + 4], f32, tag="acc")

        prev_act = None

        def chain(inst):
            nonlocal prev_act
            if prev_act is not None:
                add_dep_helper(inst.ins, prev_act.ins, sync=False,
                               reason="act phase order")
            prev_act = inst

        tt_colps = {}
        for group in _GROUPS:
            stiles = []
            for (ph, b) in group:
                tt = ph == "tt"

                def rsl(base, c0, c1):
                    if tt:
                        hf, off = ("a", 0) if c0 < H_TT else ("b", H_TT)
                        return sbt[f"{base}_tt_{hf}"][:, c0 - off:c1 - off]
                    return sbt[f"{base}_st"][:, c0:c1]

                band = BAND_TT if tt else BAND_ST
                R = ROWS_TT if tt else ROWS_ST
                blkidx = b if tt else BLKS_TT + b
                bc = slice(b * 128, (b + 1) * 128)
                pool = stt_p if tt else sst_p
                s_t = pool.tile([128, band], f16, tag="s")
                for w0 in range(0, band, WIN):
                    wlen = min(WIN, band - w0)
                    ps = pspool.tile([128, WIN], f32, tag="ps")
                    for c0 in range(w0, w0 + wlen, 512):
                        sub = ps[:, c0 - w0:c0 - w0 + 512]
                        nc.tensor.matmul(sub, rsl("r0", b * 128, (b + 1) * 128),
                                         rsl("r0", c0, c0 + 512),
                                         start=True, stop=False)
                        nc.tensor.matmul(sub, rsl("r1", b * 128, (b + 1) * 128),
                                         rsl("r1", c0, c0 + 512),
                                         start=False, stop=False)
                        # masks that land in this 512-subtile:
                        mask_mms = []
                        dg0 = b * 128          # diag block base col
                        if dg0 // 512 * 512 == c0:
                            mask_mms.append((dg0 % 512, sbt["mdiag"], 128))
                        if not tt:
                            du0 = 4096 + b * 128   # duplicate block base col
                            if du0 // 512 * 512 == c0:
                                mask_mms.append((du0 % 512, sbt["mdup"], 128))
                        if c0 >= band - R:     # antipodal block
                            mask_mms.append((0, sbt["am"], 512))
                        nc.tensor.matmul(sub, sbt["ones2"][:],
                                         rsl("augr", c0, c0 + 512),
                                         start=False, stop=not mask_mms)
                        for mi, (off, mt, mlen) in enumerate(mask_mms):
                            nc.tensor.matmul(
                                ps[:, c0 - w0 + off:c0 - w0 + off + mlen],
                                sbt["ident"][:], mt[:, 0:mlen],
                                start=False, stop=(mi == len(mask_mms) - 1))
                    # PSUM -> fp16 SBUF, adding exact fp32 -sq_i/2
                    nc.vector.tensor_scalar(
                        s_t[:, w0:w0 + wlen], ps[:, 0:wlen],
                        sbt["sqi"][:, blkidx:blkidx + 1], None,
                        op0=mybir.AluOpType.add)
                stiles.append(s_t)
            dtiles = []
            for k, (ph, b) in enumerate(group):
                tt = ph == "tt"
                band = BAND_TT if tt else BAND_ST
                pool = dtt_p if tt else dst_p
                d_t = pool.tile([128, band], f16, tag="d")
                si = nc.scalar.activation(
                    d_t[:], stiles[k][:], mybir.ActivationFunctionType.Sqrt,
                    bias=0.0, scale=-2.0)
                chain(si)
                dtiles.append(d_t)
            for k, (ph, b) in enumerate(group):
                tt = ph == "tt"
                band = BAND_TT if tt else BAND_ST
                R = ROWS_TT if tt else ROWS_ST
                blkidx = b if tt else BLKS_TT + b
                pool = ett_p if tt else est_p
                chunked = (not tt) and len(group) == 1
                if chunked:
                    # tail blocks: 2 exp chunks so colsum matmuls overlap exp
                    s_idx = b - (BLKS_ST - 2)
                    echunks = []
                    for ci, (c0, c1) in enumerate(((0, 3072), (3072, band))):
                        e_t = pool.tile([128, c1 - c0], bf16, tag="e",
                                        name=f"ec{b}_{ci}")
                        ei = nc.scalar.activation(
                            e_t[:], dtiles[k][:, c0:c1],
                            mybir.ActivationFunctionType.Exp,
                            bias=MARGIN, scale=-1.0,
                            accum_out=acc[:, NBLK + 2 * s_idx + ci:
                                          NBLK + 2 * s_idx + ci + 1])
                        chain(ei)
                        echunks.append((c0, c1, e_t))

                    def esl(c0, c1):
                        for b0, b1, t in echunks:
                            if c0 >= b0 and c1 <= b1:
                                return t[:, c0 - b0:c1 - b0]
                        raise AssertionError
                else:
                    e_t = pool.tile([128, band], bf16, tag="e")
                    ei = nc.scalar.activation(
                        e_t[:], dtiles[k][:], mybir.ActivationFunctionType.Exp,
                        bias=MARGIN, scale=-1.0,
                        accum_out=acc[:, blkidx:blkidx + 1])
                    chain(ei)

                    def esl(c0, c1):
                        return e_t[:, c0:c1]
                # column sums over cols [R, band): ones_cs.T @ E.
                # tt blocks share persistent PSUM accumulators (one drain);
                # st blocks drain per block.
                cout = out_ctt if tt else out_cst
                for q0 in range(R, band, WIN):
                    if tt:
                        if b == 0:
                            tt_colps[q0] = cpspool.tile(
                                [65, WIN], f32, tag="cps",
                                name=f"ttcps{q0}")
                        colps = tt_colps[q0]
                    else:
                        colps = cpspool.tile([65, WIN], f32, tag="cps")
                    for c0 in range(q0, q0 + WIN, 512):
                        nc.tensor.matmul(colps[:, c0 - q0:c0 - q0 + 512],
                                         sbt["ones_cs"][:],
                                         esl(c0, c0 + 512),
                                         start=(not tt or b == 0),
                                         stop=(not tt or b == BLKS_TT - 1))
                    if not tt or b == BLKS_TT - 1:
                        orow = 0 if tt else b
                        stg = cstg_p.tile([1, WIN], f32, tag="stg")
                        nc.vector.tensor_copy(stg[:], colps[0:1, :])
                        nc.sync.dma_start(
                            cout.ap()[orow:orow + 1, q0 - R:q0 - R + WIN],
                            stg[:])

        for t in range(NBLK + 4):
            nc.sync.dma_start(out_acc.ap()[t * 128:(t + 1) * 128, :],
                              acc[:, t:t + 1])

    nc.compile()
    return nc


def _get_program():
    if "nc" not in _prog_cache:
        _prog_cache["nc"] = _build_program()
    return _prog_cache["nc"]


def _prep_side(Xb):
    """Xb: [N, D] fp32 of bf16-rounded values. Returns (r0, r1, augr, sqh)."""
    XT = Xb.T  # [D, N]
    sqh = (0.5 * (Xb.astype(np.float64) ** 2).sum(axis=1)).astype(np.float32)
    h = sqh.astype(bf).astype(np.float32)
    l = (sqh - h).astype(bf).astype(np.float32)
    N = Xb.shape[0]
    augr = np.zeros((65, N), dtype=np.float32)
    augr[0] = -h
    augr[1] = -l
    return XT[0:128], XT[128:256], augr, sqh


def _build_emb(text, shape):
    N = text.shape[0]
    shape_rep = np.repeat(shape, 2, axis=0)
    mask = np.tile(np.array([1.0, 0.0], dtype=text.dtype), N // 2)[:, None]
    inv = 1.0 - mask
    e1 = text * mask + shape_rep * inv
    e2 = text * inv + shape_rep * mask
    return np.concatenate([e1, e2], axis=0)


def _pair_dists(X):
    i = np.arange(X.shape[0] // 2) * 2
    d = X[i].astype(np.float64) - X[i + 1].astype(np.float64)
    return np.sqrt((d * d).sum(axis=1))


def run(inputs, trace=False):
    text = np.asarray(inputs["text_embeddings"], dtype=np.float32)
    shape = np.asarray(inputs["shape_embeddings"], dtype=np.float32)

    tb = text.astype(bf).astype(np.float32)
    emb = _build_emb(text, shape)
    eb = emb.astype(bf).astype(np.float32)

    r0_tt, r1_tt, augr_tt, sq_tt = _prep_side(tb)
    r0_st, r1_st, augr_st, sq_st = _prep_side(eb)

    ident = np.eye(128, dtype=np.float32)
    ones2 = np.zeros((65, 128), dtype=np.float32)
    ones2[0:2] = 1.0
    ones_cs = np.ones((128, 65), dtype=np.float32)
    mdiag = np.zeros((128, 128), dtype=np.float32)
    rr = np.arange(128)
    mdiag[rr, rr] = MASK_VAL
    mdiag[rr, rr ^ 1] = MASK_VAL
    # duplicate-row masks: e1 cores (odd local row r -> col r-1),
    # e2 cores (even local row r -> col r+1)
    mdup_e1 = np.zeros((128, 128), dtype=np.float32)
    ro = rr[1::2]
    mdup_e1[ro, ro - 1] = MASK_VAL
    mdup_e2 = np.zeros((128, 128), dtype=np.float32)
    re = rr[0::2]
    mdup_e2[re, re + 1] = MASK_VAL
    am_lo = np.zeros((128, 512), dtype=np.float32)
    am_hi = np.full((128, 512), MASK_VAL, dtype=np.float32)

    in_maps = []
    for c in range(CORES):
        sqi = np.zeros((128, NBLK), dtype=np.float32)
        for b in range(BLKS_TT):
            sqi[:, b] = -sq_tt[c * ROWS_TT + b * 128:
                               c * ROWS_TT + (b + 1) * 128]
        for b in range(BLKS_ST):
            sqi[:, BLKS_TT + b] = -sq_st[c * ROWS_ST + b * 128:
                                         c * ROWS_ST + (b + 1) * 128]
        m = {
            "ones2": ones2.astype(bf),
            "ones_cs": ones_cs.astype(bf),
            "ident": ident.astype(bf),
            "mdiag": mdiag.astype(bf),
            "mdup": (mdup_e1 if c < CORES // 2 else mdup_e2).astype(bf),
            "am": (am_lo if c < CORES // 2 else am_hi).astype(bf),
            "sqi": sqi,
            "r0_tt": np.roll(r0_tt, -c * ROWS_TT, axis=1).astype(bf),
            "r1_tt": np.roll(r1_tt, -c * ROWS_TT, axis=1).astype(bf),
            "augr_tt": np.roll(augr_tt, -c * ROWS_TT, axis=1).astype(bf),
            "r0_st": np.roll(r0_st, -c * ROWS_ST, axis=1).astype(bf),
            "r1_st": np.roll(r1_st, -c * ROWS_ST, axis=1).astype(bf),
            "augr_st": np.roll(augr_st, -c * ROWS_ST, axis=1).astype(bf),
        }
        in_maps.append(m)

    nc = _get_program()
    res = run_bass_kernel_spmd(nc, in_maps, core_ids=list(range(CORES)),
                               trace=trace)

    r_tt = np.zeros(N_TT, dtype=np.float64)
    r_st = np.zeros(N_ST, dtype=np.float64)
    for c in range(CORES):
        a = res.results[c]["acc"][:, 0].astype(np.float64)
        for b in range(BLKS_TT):
            r_tt[c * ROWS_TT + b * 128:c * ROWS_TT + (b + 1) * 128] += \
                a[b * 128:(b + 1) * 128]
        for b in range(BLKS_ST):
            if b >= BLKS_ST - 2:
                s_idx = b - (BLKS_ST - 2)
                t0c = NBLK + 2 * s_idx
                col = a[t0c * 128:(t0c + 1) * 128] + \
                    a[(t0c + 1) * 128:(t0c + 2) * 128]
            else:
                t = BLKS_TT + b
                col = a[t * 128:(t + 1) * 128]
            r_st[c * ROWS_ST + b * 128:c * ROWS_ST + (b + 1) * 128] += col
        cs_tt = res.results[c]["ctt"][0].astype(np.float64)
        cs_st = res.results[c]["cst"].astype(np.float64).sum(axis=0)
        idx_tt = (c * ROWS_TT + ROWS_TT + np.arange(BAND_TT - ROWS_TT)) % N_TT
        np.add.at(r_tt, idx_tt, cs_tt)
        idx_st = (c * ROWS_ST + ROWS_ST + np.arange(BAND_ST - ROWS_ST)) % N_ST
        np.add.at(r_st, idx_st, cs_st)

    def phase_loss(rowsums, pdist, dup_add):
        P = rowsums.shape[0] // 2
        i = np.arange(P) * 2
        neg_sum = rowsums[i] + rowsums[i + 1] + dup_add
        J = np.log(neg_sum) + pdist
        Jr = np.maximum(J, 0.0)
        return (Jr * Jr).sum() / (2.0 * P)

    loss_tt = phase_loss(r_tt, _pair_dists(text), 0.0)
    loss_st = phase_loss(r_st, _pair_dists(emb), np.exp(MARGIN))
    out = np.asarray(loss_tt + loss_st, dtype=np.float32)
    if trace:
        return out, res
    return out


def kernel(**inputs):
    return run(inputs)
